# revision 11
# baseline (speedup 1.0000x reference)
"""Trainium2 Bass kernel for nn_MultiHeadAttention_53309134078537.

Reference computation (B=4, S=2048, D=512, H=8, HD=64):
    q = split_heads(Q @ wq + b); k = split_heads(K @ wq + b); v = split_heads(V @ wq + b)
    logits = (q @ k^T) / 8 + pad_mask * (-1e9)
    attn = softmax(logits)          # (B, H, S, S) -- 512 MB fp32, dominates memory traffic
    z = attn @ v; out = merge(z) @ out_kernel + out_bias
    returns (out, attn)

Sharding: 8 cores = (batch b = c//2) x (head-group hg = c%2, 4 heads each).
Per core, attention is computed transposed (logitsT[k, q]) so the padding mask is a
per-partition ACT bias and the z matmul needs no on-chip transpose of the 16.8M-element
attention matrix. Softmax column-sums are accumulated early by M=1 ones-matmuls packed
into the four PE column-groups (tile_position), so 1/sum (exp(-ln s) on ACT) and the
DVE normalization of attn^T overlap the next head's logits/exp instead of serializing
behind z. z then consumes the already-normalized attn^T. Heads are software-pipelined;
attn is written to HBM as fp16 [k, q]; the host transposes to [q, k] and casts to fp32
(pure data movement). The out-projection partials of the two head-groups are summed on
the host (out_bias is passed as zeros to the hg=1 cores).
"""

import numpy as np

B, S, D, H, HD = 4, 2048, 512, 8, 64
HPC = 4            # heads per core
DHG = HPC * HD     # 256: d_out slice per core
NCORES = 8
SCALE = 1.0 / 8.0
NEG = -1e9 * SCALE  # mask bias applied after the activation scale

P = 128
ST = S // P        # 16 tiles of 128 along sequence
QC = S // 512      # 4 q-chunks of 512
DI = D // P        # 4 tiles of 128 along d_in
DO2 = DHG // P     # 2 tiles of 128 along the core's d_out slice

_CACHE = {}


def _build():
    import concourse.bass as bass
    import concourse.tile as tile
    from concourse import bacc, mybir

    f32, f16 = mybir.dt.float32, mybir.dt.float16
    AF = mybir.ActivationFunctionType
    ALU = mybir.AluOpType

    nc = bacc.Bacc("TRN2", target_bir_lowering=False)

    Q = nc.dram_tensor("q_in", [S, D], f32, kind="ExternalInput")
    K = nc.dram_tensor("k_in", [S, D], f32, kind="ExternalInput")
    V = nc.dram_tensor("v_in", [S, D], f32, kind="ExternalInput")
    MASK = nc.dram_tensor("mask", [1, S], f32, kind="ExternalInput")
    WQ = nc.dram_tensor("wq", [D, DHG], f32, kind="ExternalInput")
    WQB = nc.dram_tensor("wqb", [1, DHG], f32, kind="ExternalInput")
    WO = nc.dram_tensor("wo", [DHG, D], f32, kind="ExternalInput")
    WOB = nc.dram_tensor("wob", [1, D], f32, kind="ExternalInput")
    ATTN = nc.dram_tensor("attn_t", [HPC, S, S], f16, kind="ExternalOutput")
    OUT = nc.dram_tensor("out_p", [S, D], f32, kind="ExternalOutput")

    with tile.TileContext(nc) as tc:
        with (
            tc.tile_pool(name="persist", bufs=1) as persist,
            tc.tile_pool(name="dram", bufs=1, space="DRAM") as dram,
            tc.tile_pool(name="psL", bufs=2, space="PSUM") as psL,
            tc.tile_pool(name="psZ", bufs=2, space="PSUM") as psZ,
            tc.tile_pool(name="psS", bufs=2, space="PSUM") as psS,
            tc.tile_pool(name="work", bufs=2) as work,
        ):
            # ---- persistent SBUF state ----
            qT = persist.tile([P, DO2, S], f16)      # q_projT: [dout, s]
            kT = persist.tile([P, DO2, S], f16)      # k_projT
            zT = persist.tile([P, DO2, S], f16)      # zT: rows h*64..h*64+64 per head
            vp = persist.tile([P, ST, DHG], f16)     # v_proj, natural [s, dout]
            wo_sb = persist.tile([P, DO2, D], f16)
            ones_col = persist.tile([P, 1], f16)
            mask_bias = persist.tile([P, ST], f32)   # NEG * mask, partition layout
            wqb_part = persist.tile([P, DO2], f32)   # wq bias, partition layout
            wqb_bc = persist.tile([P, DHG], f16)     # wq bias broadcast along partitions
            wob_bc = persist.tile([P, D], f16)       # out bias broadcast along partitions

            nc.vector.memset(ones_col, 1.0)

            with tc.tile_pool(name="load", bufs=1) as load:
                # fp16 Q in DRAM first: its transpose + projection is the
                # critical path to the first logits matmul.
                x16s = []
                for i, src in enumerate((Q, K, V)):
                    x16 = dram.tile([S, D], f16, tag="x16", bufs=3, name=f"x16_{i}")
                    nc.gpsimd.dma_start(out=x16, in_=src.ap())
                    x16s.append(x16)

                # weights on the HWDGE queues + DVE cast (keeps SWDGE free)
                wq_f32 = load.tile([P, DI, DHG], f32)
                nc.sync.dma_start(
                    out=wq_f32, in_=WQ.ap().rearrange("(t p) n -> p t n", p=P)
                )
                wq_sb = load.tile([P, DI, DHG], f16)
                nc.vector.tensor_copy(out=wq_sb, in_=wq_f32)
                wo_f32 = load.tile([P, DO2, D], f32)
                nc.scalar.dma_start(
                    out=wo_f32, in_=WO.ap().rearrange("(t p) n -> p t n", p=P)
                )
                nc.vector.tensor_copy(out=wo_sb, in_=wo_f32)
                nc.sync.dma_start(
                    out=wqb_part, in_=WQB.ap().rearrange("1 (t p) -> p t", p=P)
                )
                nc.gpsimd.dma_start(out=wqb_bc, in_=WQB.ap().to_broadcast((P, DHG)))
                nc.gpsimd.dma_start(out=wob_bc, in_=WOB.ap().to_broadcast((P, D)))

                mask_part = load.tile([P, ST], f32)
                nc.sync.dma_start(
                    out=mask_part, in_=MASK.ap().rearrange("1 (t p) -> p t", p=P)
                )
                nc.vector.tensor_scalar_mul(out=mask_bias, in0=mask_part, scalar1=NEG)

                # Per tensor: xbar-transpose the fp16 copy to X^T, then project.
                for i, dst in ((0, qT), (1, kT), (2, None)):
                    xTsb = load.tile([P, DI, S], f16, tag="xT", bufs=2, name=f"xT{i}")
                    for t in range(DI):
                        nc.sync.dma_start_transpose(
                            out=xTsb[:, t, :], in_=x16s[i][:, t * P : (t + 1) * P]
                        )
                    if dst is not None:
                        for dot in range(DO2):
                            for qc in range(QC):
                                ps = psL.tile([P, 1024], f32, tag="l")
                                for di in range(DI):
                                    nc.tensor.matmul(
                                        ps[:, :512],
                                        lhsT=wq_sb[:, di, dot * P : (dot + 1) * P],
                                        rhs=xTsb[:, di, qc * 512 : (qc + 1) * 512],
                                        start=(di == 0),
                                        stop=(di == DI - 1),
                                    )
                                nc.vector.tensor_scalar(
                                    out=dst[:, dot, qc * 512 : (qc + 1) * 512],
                                    in0=ps[:, :512],
                                    scalar1=wqb_part[:, dot : dot + 1],
                                    scalar2=None,
                                    op0=ALU.add,
                                )
                    else:
                        for st in range(ST):
                            ps = psL.tile([P, 1024], f32, tag="l")
                            for di in range(DI):
                                nc.tensor.matmul(
                                    ps[:, :DHG],
                                    lhsT=xTsb[:, di, st * P : (st + 1) * P],
                                    rhs=wq_sb[:, di, :],
                                    start=(di == 0),
                                    stop=(di == DI - 1),
                                )
                            nc.vector.tensor_add(
                                out=vp[:, st, :], in0=ps[:, :DHG], in1=wqb_bc
                            )

            # ---- attention: heads software-pipelined ----
            with tc.tile_pool(name="exp", bufs=34) as exp_pool:
                expT_all = {}

                def emit_head(h):
                    """logits + exp + packed column-sums for head h."""
                    dot, r0 = h // 2, (h % 2) * HD
                    tiles = []
                    sum_ps = psS.tile([P, 512], f32, tag="s", name=f"sum_{h}")
                    for kt in range(ST):
                        et = exp_pool.tile(
                            [P, S], f16, tag="expT", name=f"expT_{h}_{kt}"
                        )
                        tiles.append(et)
                        for half in range(2):
                            ps = psL.tile([P, 1024], f32, tag="l")
                            for j in range(2):
                                c0 = half * 1024 + j * 512
                                nc.tensor.matmul(
                                    ps[:, j * 512 : (j + 1) * 512],
                                    lhsT=kT[r0 : r0 + HD, dot, kt * P : (kt + 1) * P],
                                    rhs=qT[r0 : r0 + HD, dot, c0 : c0 + 512],
                                    start=True,
                                    stop=True,
                                )
                            nc.scalar.activation(
                                out=et[:, half * 1024 : (half + 1) * 1024],
                                in_=ps,
                                func=AF.Exp,
                                bias=mask_bias[:, kt : kt + 1],
                                scale=SCALE,
                            )
                        # column sums: 4 concurrent M=1 matmuls, one per PE
                        # column-group, accumulated over k tiles
                        for qc in range(QC):
                            nc.tensor.matmul(
                                sum_ps[32 * qc : 32 * qc + 1, :],
                                lhsT=ones_col,
                                rhs=et[:, qc * 512 : (qc + 1) * 512],
                                start=(kt == 0),
                                stop=(kt == ST - 1),
                                tile_position=(0, 32 * qc),
                            )
                    expT_all[h] = (tiles, sum_ps)

                def emit_norm(h):
                    """1/sum + in-place normalize + attn writeback for head h."""
                    tiles, sum_ps = expT_all[h]
                    rln = work.tile([P, 512], f32, tag="rln")
                    rsum_dram = dram.tile([1, S], f32, tag="rsum_d", bufs=2)
                    for qc in range(QC):
                        r = slice(32 * qc, 32 * qc + 1)
                        nc.scalar.activation(out=rln[r, :], in_=sum_ps[r, :], func=AF.Ln)
                        nc.scalar.activation(
                            out=rln[r, :], in_=rln[r, :], func=AF.Exp, scale=-1.0
                        )
                        nc.sync.dma_start(
                            out=rsum_dram[:, qc * 512 : (qc + 1) * 512], in_=rln[r, :]
                        )
                    rbc = work.tile([P, S], f16, tag="rbc")
                    nc.gpsimd.dma_start(out=rbc, in_=rsum_dram.to_broadcast((P, S)))
                    for kt in range(ST):
                        nc.vector.tensor_mul(out=tiles[kt], in0=tiles[kt], in1=rbc)
                        nc.sync.dma_start(
                            out=ATTN.ap()[h, kt * P : (kt + 1) * P, :], in_=tiles[kt]
                        )

                def emit_z(h):
                    """z^T from the normalized attn^T; writes zT in fp16."""
                    dot, r0 = h // 2, (h % 2) * HD
                    tiles, _ = expT_all.pop(h)
                    for qc in range(QC):
                        ps = psZ.tile([P, 512], f32, tag="z")
                        for kt in range(ST):
                            nc.tensor.matmul(
                                ps[:HD, :],
                                lhsT=vp[:, kt, r0 + (dot * P) : r0 + (dot * P) + HD],
                                rhs=tiles[kt][:, qc * 512 : (qc + 1) * 512],
                                start=(kt == 0),
                                stop=(kt == ST - 1),
                            )
                        nc.vector.tensor_copy(
                            out=zT[r0 : r0 + HD, dot, qc * 512 : (qc + 1) * 512],
                            in_=ps[:HD, :],
                        )

                for h in range(HPC):
                    emit_head(h)
                    emit_norm(h)
                    if h > 0:
                        emit_z(h - 1)
                emit_z(HPC - 1)

                # ---- out projection ----
                for qt in range(ST):
                    ps = psL.tile([P, 1024], f32, tag="l")
                    for dt in range(DO2):
                        nc.tensor.matmul(
                            ps[:, :512],
                            lhsT=zT[:, dt, qt * P : (qt + 1) * P],
                            rhs=wo_sb[:, dt, :],
                            start=(dt == 0),
                            stop=(dt == DO2 - 1),
                        )
                    osb = work.tile([P, D], f32, tag="osb")
                    nc.vector.tensor_add(out=osb, in0=ps[:, :512], in1=wob_bc)
                    nc.sync.dma_start(out=OUT.ap()[qt * P : (qt + 1) * P, :], in_=osb)

    nc.finalize()
    return nc


def kernel(Q, K, V, pad_mask, wq_kernel, wq_bias, out_kernel, out_bias, **run_kwargs):
    from concourse.bass_utils import run_bass_kernel_spmd

    if "nc" not in _CACHE:
        _CACHE["nc"] = _build()
    nc = _CACHE["nc"]

    in_maps = []
    for c in range(NCORES):
        b, hg = c // 2, c % 2
        hs = slice(hg * DHG, (hg + 1) * DHG)
        in_maps.append(
            {
                "q_in": np.ascontiguousarray(Q[b], dtype=np.float32),
                "k_in": np.ascontiguousarray(K[b], dtype=np.float32),
                "v_in": np.ascontiguousarray(V[b], dtype=np.float32),
                "mask": np.ascontiguousarray(
                    pad_mask[b, 0, 0, :][None, :], dtype=np.float32
                ),
                "wq": np.ascontiguousarray(wq_kernel[:, hs], dtype=np.float32),
                "wqb": np.ascontiguousarray(wq_bias[hs][None, :], dtype=np.float32),
                "wo": np.ascontiguousarray(out_kernel[hs, :], dtype=np.float32),
                "wob": np.ascontiguousarray(
                    (out_bias if hg == 0 else np.zeros_like(out_bias))[None, :],
                    dtype=np.float32,
                ),
            }
        )

    res = run_bass_kernel_spmd(nc, in_maps, core_ids=list(range(NCORES)), **run_kwargs)
    results = res.results if hasattr(res, "results") else res

    out = np.empty((B, S, D), dtype=np.float32)
    attn = np.empty((B, H, S, S), dtype=np.float32)
    for c in range(NCORES):
        b, hg = c // 2, c % 2
        at = results[c]["attn_t"]  # fp16 [HPC, S(k), S(q)]
        for i in range(HPC):
            attn[b, hg * HPC + i] = at[i].T
    for b in range(B):
        out[b] = results[2 * b]["out_p"] + results[2 * b + 1]["out_p"]
    if "trace" in run_kwargs:
        _CACHE["last_run"] = res
    return out, attn


# revision 12
# speedup vs baseline: 1.0801x; 1.0801x over previous
"""Trainium2 Bass kernel for nn_MultiHeadAttention_53309134078537.

Reference computation (B=4, S=2048, D=512, H=8, HD=64):
    q = split_heads(Q @ wq + b); k = split_heads(K @ wq + b); v = split_heads(V @ wq + b)
    logits = (q @ k^T) / 8 + pad_mask * (-1e9)
    attn = softmax(logits)          # (B, H, S, S) -- 512 MB fp32, dominates memory traffic
    z = attn @ v; out = merge(z) @ out_kernel + out_bias
    returns (out, attn)

Sharding: 8 cores = (batch b = c//2) x (head-group hg = c%2, 4 heads each).

Per core the attention is computed transposed (logitsT[k, q]): the padding mask becomes a
per-partition ACT bias and the z matmul consumes attn^T directly (no on-chip transpose of
the 16.8M-element attention matrix). Softmax sums ride along as a ones-column in the v
operand of the z matmul; 1/sum = exp(-ln(sum)) on ACT, broadcast across partitions via a
small DRAM bounce, applied in-place by DVE in 16-bit 2x mode; z is renormalized by the
same broadcast row. Work is pipelined in HALF-HEAD units (k=2048, q=1024): unit tiles are
small enough to keep ~3 units in flight, which hides the per-unit z -> 1/sum -> normalize
tail behind the next units' logits/exp and keeps PE/ACT dense (HAM stays warm).

attn is written to HBM as fp16 [k, q]; the host transposes to [q, k] and casts to fp32
(pure data movement). The out-projection partials of the two head-groups are summed on
the host (out_bias is passed as zeros to the hg=1 cores).
"""

import numpy as np

B, S, D, H, HD = 4, 2048, 512, 8, 64
HPC = 4            # heads per core
DHG = HPC * HD     # 256: d_out slice per core
NCORES = 8
SCALE = 1.0 / 8.0
NEG = -1e9 * SCALE  # mask bias applied after the activation scale

P = 128
ST = S // P        # 16 tiles of 128 along sequence (k)
QH = S // 2        # 1024: q extent of one unit
DI = D // P        # 4 tiles of 128 along d_in
DO2 = DHG // P     # 2 tiles of 128 along the core's d_out slice

_CACHE = {}


def _build():
    import concourse.bass as bass
    import concourse.tile as tile
    from concourse import bacc, mybir

    f32, f16 = mybir.dt.float32, mybir.dt.float16
    AF = mybir.ActivationFunctionType
    ALU = mybir.AluOpType

    nc = bacc.Bacc("TRN2", target_bir_lowering=False)

    Q = nc.dram_tensor("q_in", [S, D], f32, kind="ExternalInput")
    K = nc.dram_tensor("k_in", [S, D], f32, kind="ExternalInput")
    V = nc.dram_tensor("v_in", [S, D], f32, kind="ExternalInput")
    MASK = nc.dram_tensor("mask", [1, S], f32, kind="ExternalInput")
    WQ = nc.dram_tensor("wq", [D, DHG], f32, kind="ExternalInput")
    WQB = nc.dram_tensor("wqb", [1, DHG], f32, kind="ExternalInput")
    WO = nc.dram_tensor("wo", [DHG, D], f32, kind="ExternalInput")
    WOB = nc.dram_tensor("wob", [1, D], f32, kind="ExternalInput")
    ATTN = nc.dram_tensor("attn_t", [HPC, S, S], f16, kind="ExternalOutput")
    OUT = nc.dram_tensor("out_p", [S, D], f32, kind="ExternalOutput")

    with tile.TileContext(nc) as tc:
        with (
            tc.tile_pool(name="persist", bufs=1) as persist,
            tc.tile_pool(name="dram", bufs=1, space="DRAM") as dram,
            tc.tile_pool(name="psL", bufs=3, space="PSUM") as psL,
            tc.tile_pool(name="psZ", bufs=2, space="PSUM") as psZ,
            tc.tile_pool(name="work", bufs=2) as work,
        ):
            # ---- persistent SBUF state ----
            qT = persist.tile([P, DO2, S], f16)      # q_projT: [dout, s]
            kT = persist.tile([P, DO2, S], f16)      # k_projT
            zT = persist.tile([P, DO2, S], f16)      # zT: rows h*64..h*64+64 per head
            vext = persist.tile([P, ST, HPC, HD + 1], f16)  # v_proj + ones column
            wo_sb = persist.tile([P, DO2, D], f16)
            mask_bias = persist.tile([P, ST], f32)   # NEG * mask, partition layout
            wqb_part = persist.tile([P, DO2], f32)   # wq bias, partition layout
            wqb_bc = persist.tile([P, DHG], f16)     # wq bias broadcast along partitions
            wob_bc = persist.tile([P, D], f16)       # out bias broadcast along partitions

            with tc.tile_pool(name="load", bufs=1) as load:
                # fp16 copies of Q/K/V in DRAM (xbar transpose is 16-bit only)
                x16s = []
                for i, src in enumerate((Q, K, V)):
                    x16 = dram.tile([S, D], f16, tag="x16", bufs=3, name=f"x16_{i}")
                    nc.gpsimd.dma_start(out=x16, in_=src.ap())
                    x16s.append(x16)

                # weights via the HWDGE queues + DVE cast (keeps SWDGE free)
                wq_f32 = load.tile([P, DI, DHG], f32)
                nc.sync.dma_start(
                    out=wq_f32, in_=WQ.ap().rearrange("(t p) n -> p t n", p=P)
                )
                wq_sb = load.tile([P, DI, DHG], f16)
                nc.vector.tensor_copy(out=wq_sb, in_=wq_f32)
                wo_f32 = load.tile([P, DO2, D], f32)
                nc.scalar.dma_start(
                    out=wo_f32, in_=WO.ap().rearrange("(t p) n -> p t n", p=P)
                )
                nc.vector.tensor_copy(out=wo_sb, in_=wo_f32)
                nc.sync.dma_start(
                    out=wqb_part, in_=WQB.ap().rearrange("1 (t p) -> p t", p=P)
                )
                nc.gpsimd.dma_start(out=wqb_bc, in_=WQB.ap().to_broadcast((P, DHG)))
                nc.gpsimd.dma_start(out=wob_bc, in_=WOB.ap().to_broadcast((P, D)))

                mask_part = load.tile([P, ST], f32)
                nc.sync.dma_start(
                    out=mask_part, in_=MASK.ap().rearrange("1 (t p) -> p t", p=P)
                )
                nc.vector.tensor_scalar_mul(out=mask_bias, in0=mask_part, scalar1=NEG)

                nc.vector.memset(vext, 0.0)

                # Per tensor: xbar-transpose the fp16 copy to X^T in SBUF, then
                # project. q/k produce [dout, s]; v lands in per-head v_ext tiles.
                for i, dst in ((0, qT), (1, kT), (2, None)):
                    xTsb = load.tile([P, DI, S], f16, tag="xT", bufs=2, name=f"xT{i}")
                    for t in range(DI):
                        nc.sync.dma_start_transpose(
                            out=xTsb[:, t, :], in_=x16s[i][:, t * P : (t + 1) * P]
                        )
                    if dst is not None:
                        for dot in range(DO2):
                            for qc in range(S // 512):
                                ps = psL.tile([P, 1024], f32, tag="l")
                                for di in range(DI):
                                    nc.tensor.matmul(
                                        ps[:, :512],
                                        lhsT=wq_sb[:, di, dot * P : (dot + 1) * P],
                                        rhs=xTsb[:, di, qc * 512 : (qc + 1) * 512],
                                        start=(di == 0),
                                        stop=(di == DI - 1),
                                    )
                                nc.vector.tensor_scalar(
                                    out=dst[:, dot, qc * 512 : (qc + 1) * 512],
                                    in0=ps[:, :512],
                                    scalar1=wqb_part[:, dot : dot + 1],
                                    scalar2=None,
                                    op0=ALU.add,
                                )
                    else:
                        for st in range(ST):
                            ps = psL.tile([P, 1024], f32, tag="l")
                            for di in range(DI):
                                nc.tensor.matmul(
                                    ps[:, :DHG],
                                    lhsT=xTsb[:, di, st * P : (st + 1) * P],
                                    rhs=wq_sb[:, di, :],
                                    start=(di == 0),
                                    stop=(di == DI - 1),
                                )
                            for h in range(HPC):
                                nc.vector.tensor_add(
                                    out=vext[:, st, h, :HD],
                                    in0=ps[:, h * HD : (h + 1) * HD],
                                    in1=wqb_bc[:, h * HD : (h + 1) * HD],
                                )
                        nc.vector.memset(vext[:, :, :, HD : HD + 1], 1.0)

            # ---- attention: 8 half-head units, software-pipelined ----
            with tc.tile_pool(name="exp", bufs=48) as exp_pool:
                unit_state = {}

                def emit_logits_exp(u):
                    h, qh = u // 2, u % 2
                    dot, r0 = h // 2, (h % 2) * HD
                    q0 = qh * QH
                    tiles = []
                    for kt in range(ST):
                        et = exp_pool.tile(
                            [P, QH], f16, tag="expT", name=f"expT_{u}_{kt}"
                        )
                        tiles.append(et)
                        ps = psL.tile([P, 1024], f32, tag="l")
                        for j in range(2):
                            nc.tensor.matmul(
                                ps[:, j * 512 : (j + 1) * 512],
                                lhsT=kT[r0 : r0 + HD, dot, kt * P : (kt + 1) * P],
                                rhs=qT[r0 : r0 + HD, dot, q0 + j * 512 : q0 + (j + 1) * 512],
                                start=True,
                                stop=True,
                            )
                        nc.scalar.activation(
                            out=et,
                            in_=ps,
                            func=AF.Exp,
                            bias=mask_bias[:, kt : kt + 1],
                            scale=SCALE,
                        )
                    unit_state[u] = tiles

                def emit_tail(u):
                    h, qh = u // 2, u % 2
                    dot, r0 = h // 2, (h % 2) * HD
                    q0 = qh * QH
                    tiles = unit_state.pop(u)

                    # z^T (+ sums row 64) over k; zext rows: 0..63 z, 64 sum
                    # (later 1/sum), 96 ln(sum)
                    zext = work.tile([P, QH], f32, tag="zext")
                    for qc in range(2):
                        ps = psZ.tile([P, 512], f32, tag="z")
                        for kt in range(ST):
                            nc.tensor.matmul(
                                ps[: HD + 1, :],
                                lhsT=vext[:, kt, h, :],
                                rhs=tiles[kt][:, qc * 512 : (qc + 1) * 512],
                                start=(kt == 0),
                                stop=(kt == ST - 1),
                            )
                        nc.vector.tensor_copy(
                            out=zext[: HD + 1, qc * 512 : (qc + 1) * 512],
                            in_=ps[: HD + 1, :],
                        )
                    nc.scalar.activation(
                        out=zext[96:97, :], in_=zext[HD : HD + 1, :], func=AF.Ln
                    )
                    nc.scalar.activation(
                        out=zext[HD : HD + 1, :],
                        in_=zext[96:97, :],
                        func=AF.Exp,
                        scale=-1.0,
                    )
                    rsum_dram = dram.tile([1, QH], f32, tag="rsum_d", bufs=3)
                    nc.sync.dma_start(out=rsum_dram, in_=zext[HD : HD + 1, :])
                    rbc = work.tile([P, QH], f16, tag="rbc", bufs=3)
                    nc.gpsimd.dma_start(out=rbc, in_=rsum_dram.to_broadcast((P, QH)))

                    for kt in range(ST):
                        nc.vector.tensor_mul(out=tiles[kt], in0=tiles[kt], in1=rbc)
                        nc.sync.dma_start(
                            out=ATTN.ap()[h, kt * P : (kt + 1) * P, q0 : q0 + QH],
                            in_=tiles[kt],
                        )
                    nc.vector.tensor_mul(
                        out=zT[r0 : r0 + HD, dot, q0 : q0 + QH],
                        in0=zext[:HD, :],
                        in1=rbc[:HD, :],
                    )

                for u in range(2 * HPC):
                    emit_logits_exp(u)
                    if u > 0:
                        emit_tail(u - 1)
                emit_tail(2 * HPC - 1)

                # ---- out projection ----
                for qt in range(ST):
                    ps = psL.tile([P, 1024], f32, tag="l")
                    for dt in range(DO2):
                        nc.tensor.matmul(
                            ps[:, :512],
                            lhsT=zT[:, dt, qt * P : (qt + 1) * P],
                            rhs=wo_sb[:, dt, :],
                            start=(dt == 0),
                            stop=(dt == DO2 - 1),
                        )
                    osb = work.tile([P, D], f32, tag="osb")
                    nc.vector.tensor_add(out=osb, in0=ps[:, :512], in1=wob_bc)
                    nc.sync.dma_start(out=OUT.ap()[qt * P : (qt + 1) * P, :], in_=osb)

    nc.finalize()
    return nc


def kernel(Q, K, V, pad_mask, wq_kernel, wq_bias, out_kernel, out_bias, **run_kwargs):
    from concourse.bass_utils import run_bass_kernel_spmd

    if "nc" not in _CACHE:
        _CACHE["nc"] = _build()
    nc = _CACHE["nc"]

    in_maps = []
    for c in range(NCORES):
        b, hg = c // 2, c % 2
        hs = slice(hg * DHG, (hg + 1) * DHG)
        in_maps.append(
            {
                "q_in": np.ascontiguousarray(Q[b], dtype=np.float32),
                "k_in": np.ascontiguousarray(K[b], dtype=np.float32),
                "v_in": np.ascontiguousarray(V[b], dtype=np.float32),
                "mask": np.ascontiguousarray(
                    pad_mask[b, 0, 0, :][None, :], dtype=np.float32
                ),
                "wq": np.ascontiguousarray(wq_kernel[:, hs], dtype=np.float32),
                "wqb": np.ascontiguousarray(wq_bias[hs][None, :], dtype=np.float32),
                "wo": np.ascontiguousarray(out_kernel[hs, :], dtype=np.float32),
                "wob": np.ascontiguousarray(
                    (out_bias if hg == 0 else np.zeros_like(out_bias))[None, :],
                    dtype=np.float32,
                ),
            }
        )

    res = run_bass_kernel_spmd(nc, in_maps, core_ids=list(range(NCORES)), **run_kwargs)
    results = res.results if hasattr(res, "results") else res

    out = np.empty((B, S, D), dtype=np.float32)
    attn = np.empty((B, H, S, S), dtype=np.float32)
    for c in range(NCORES):
        b, hg = c // 2, c % 2
        at = results[c]["attn_t"]  # fp16 [HPC, S(k), S(q)]
        for i in range(HPC):
            attn[b, hg * HPC + i] = at[i].T
    for b in range(B):
        out[b] = results[2 * b]["out_p"] + results[2 * b + 1]["out_p"]
    if "trace" in run_kwargs:
        _CACHE["last_run"] = res
    return out, attn
